# revision 28
# baseline (speedup 1.0000x reference)
"""WaveNet-like dense CNN on 8 TRN2 NeuronCores — batch data parallel.

Self-contained: hardcodes shapes from the problem spec.
  x: (32, 2048) f32 -> out: (32, 256) f32
Layout per core (4 batch samples): partitions p = b*32 + tc*8 + c
(b in 0..3 sample, tc in 0..3 time-chunk of 512, c in 0..7 channel),
free axis = t_low in 0..511.  All 1x1 convs are block-diagonal 64x64
strip matmuls (2 concurrent tile_position strips).  The dilated causal
conv is 3 matmuls (full tap1, shifted-view tap0, cross-chunk boundary
tap0 with a partition-shifting weight) — no data movement for shifts.
Only skip[:, :, -1] is ever used, so skip convs run on 1 column/block.
"""

import sys

sys.path.insert(0, "/opt/trn_rl_repo")

import numpy as np

import concourse.bass as bass
import concourse.tile as tile
from concourse import mybir
from concourse.bass_utils import run_bass_kernel_spmd

F32 = mybir.dt.float32
F32R = mybir.dt.float32r
AF = mybir.ActivationFunctionType
ALU = mybir.AluOpType

SEQ_LEN = 2048
C = 8
SK = 256
NB = 32
B = 32
N_CORES = 8
BL = B // N_CORES      # 4 samples per core
TCH = 4                # time chunks per sample
L = SEQ_LEN // TCH     # 512 free elems
NSTRIP = 2             # two 64x64 tile_position strips
SW = 64                # strip width (partitions/outputs per strip)
GPS = 8                # groups per strip (8 groups of 8 channels)


def _dil(i):
    return 2 ** (i % 8)


# ---------------------------------------------------------------- build


def _build_nc(fixup=True):
    nc = bass.Bass("TRN2", target_bir_lowering=False, debug=False,
                   num_devices=N_CORES)

    def din(name, shape, dt=F32R):
        return nc.dram_tensor(name, shape, dt, kind="ExternalInput").ap()

    x_d = din("x", [BL * TCH, L])
    win_d = din("win", [BL * TCH, 128])
    wd1_d = din("wd1", [128, NB * 128])
    wd0_d = din("wd0", [128, NB * 128])
    wd0s_d = din("wd0s", [128, NB * 128])
    wf_d = din("wf", [128, NB * 128])
    wg_d = din("wg", [128, NB * 128])
    wo_d = din("wo", [128, NB * 128])
    bin_d = din("bin", [128, 1], F32)
    bd_d = din("bd", [128, NB], F32)
    bf_d = din("bf", [128, NB], F32)
    bg_d = din("bg", [128, NB], F32)
    bo_d = din("bo", [128, NB], F32)
    wsk_d = din("wsk", [32, 16 * 128])
    ident_d = din("ident", [128, 128])
    bsk_d = din("bsk", [128, 2], F32)
    wo1_d = din("wo1", [128, 4 * 128])
    bo1_d = din("bo1", [128, 2], F32)
    wo2_d = din("wo2", [128, 4 * 128])
    bo2_d = din("bo2", [128, 2], F32)

    out_d = nc.dram_tensor("out", [BL, SK], F32, kind="ExternalOutput").ap()

    with tile.TileContext(nc) as tc:
        _emit(nc, tc, x_d, win_d, wd1_d, wd0_d, wd0s_d, wf_d, wg_d, wo_d,
              bin_d, bd_d, bf_d, bg_d, bo_d, wsk_d, ident_d, bsk_d,
              wo1_d, bo1_d, wo2_d, bo2_d, out_d)

    if fixup:
        _split_excess_waits(nc)
    return nc


def _emit(nc, tc, x_d, win_d, wd1_d, wd0_d, wd0s_d, wf_d, wg_d, wo_d,
          bin_d, bd_d, bf_d, bg_d, bo_d, wsk_d, ident_d, bsk_d,
          wo1_d, bo1_d, wo2_d, bo2_d, out_d):
    from contextlib import ExitStack
    ctx = ExitStack()
    const = ctx.enter_context(tc.tile_pool(name="const", bufs=1))
    work = ctx.enter_context(tc.tile_pool(name="work", bufs=2))
    hpool = ctx.enter_context(tc.tile_pool(name="h", bufs=3))
    pspool = ctx.enter_context(tc.tile_pool(name="ps", bufs=6, space="PSUM"))
    pspool1 = ctx.enter_context(tc.tile_pool(name="ps1", bufs=1, space="PSUM"))

    # ---- persistent loads
    x_sb = const.tile([BL * TCH, L], F32R, tag="x")
    nc.sync.dma_start(x_sb[:], x_d[:])
    win_sb = const.tile([BL * TCH, 128], F32R, tag="win")
    nc.sync.dma_start(win_sb[:], win_d[:])

    wtiles = {}
    for nm, d in (("wd1", wd1_d), ("wd0", wd0_d), ("wd0s", wd0s_d),
                  ("wf", wf_d), ("wg", wg_d), ("wo", wo_d)):
        t = const.tile([128, NB * 128], F32R, tag=nm)
        # per-block DMA granularity so early blocks start before all
        # weights have landed
        for i in range(NB):
            nc.sync.dma_start(t[:, 128 * i:128 * (i + 1)],
                              d[:, 128 * i:128 * (i + 1)])
        wtiles[nm] = t

    btiles = {}
    for nm, d, w in (("bin", bin_d, 1), ("bd", bd_d, NB), ("bf", bf_d, NB),
                     ("bg", bg_d, NB), ("bo", bo_d, NB), ("bsk", bsk_d, 2),
                     ("bo1", bo1_d, 2), ("bo2", bo2_d, 2)):
        t = const.tile([128, w], F32, tag=nm)
        nc.sync.dma_start(t[:], d[:])
        btiles[nm] = t

    wsk_sb = const.tile([32, 16 * 128], F32R, tag="wsk")
    nc.sync.dma_start(wsk_sb[:], wsk_d[:])
    ident_sb = const.tile([128, 128], F32R, tag="ident")
    nc.sync.dma_start(ident_sb[:], ident_d[:])
    wo1_sb = const.tile([128, 4 * 128], F32R, tag="wo1")
    nc.sync.dma_start(wo1_sb[:], wo1_d[:])
    wo2_sb = const.tile([128, 4 * 128], F32R, tag="wo2")
    nc.sync.dma_start(wo2_sb[:], wo2_d[:])

    s_sb = const.tile([128, NB], F32R, tag="scap")

    # ---- input 1x1 conv: h0 = w_in * x + b_in  (K=16 matmul broadcast)
    ps_h = pspool.tile([128, L], F32, tag="ps")
    nc.tensor.matmul(ps_h[:], win_sb[:], x_sb[:], start=True, stop=True)
    h_cur = hpool.tile([128, L], F32R, tag="h")
    nc.scalar.activation(h_cur[:], ps_h[:], AF.Identity,
                         bias=btiles["bin"][:, 0:1])

    # ---- 32 residual blocks
    for i in range(NB):
        d = _dil(i)
        wc = slice(128 * i, 128 * (i + 1))

        ps_a = pspool.tile([128, L], F32, tag="ps")
        # tap1 (current sample) over all columns
        nc.tensor.matmul(ps_a[:], wtiles["wd1"][:, wc], h_cur[:],
                         start=True, stop=False)
        if d == 1:
            # fp32r MMs need even column counts and 8B-aligned column
            # starts, so the d=1 case gets even-sized MMs plus 2 fixups.
            nc.tensor.matmul(ps_a[:, 2:L], wtiles["wd0"][:, wc],
                             h_cur[:, 1:L - 1], start=False, stop=True)
            ps_bc = pspool.tile([128, 4], F32, tag="ps")
            nc.tensor.matmul(ps_bc[:, 0:2], wtiles["wd0s"][:, wc],
                             h_cur[:, L - 2:L], start=True, stop=True,
                             skip_group_check=True)
            nc.tensor.matmul(ps_bc[:, 2:4], wtiles["wd0"][:, wc],
                             h_cur[:, 0:2], start=True, stop=True,
                             skip_group_check=True)
            # col0 += shifted-tap(prev chunk last col); col1 += tap0(h[0])
            fix_sb = work.tile([128, 2], F32, tag="fix")
            nc.scalar.copy(fix_sb[:, 0:1], ps_bc[:, 1:2])
            nc.scalar.copy(fix_sb[:, 1:2], ps_bc[:, 2:3])
            nc.vector.tensor_tensor(ps_a[:, 0:2], ps_a[:, 0:2],
                                    fix_sb[:], op=ALU.add)
        else:
            # tap0 intra-chunk: out cols [d:512] <- h cols [0:512-d]
            nc.tensor.matmul(ps_a[:, d:L], wtiles["wd0"][:, wc],
                             h_cur[:, 0:L - d], start=False, stop=False)
            # tap0 cross-chunk boundary: out cols [0:d] <- prev chunk tail
            # (weight maps group tc-1 -> tc; tc=0 groups get zero pad)
            nc.tensor.matmul(ps_a[:, 0:d], wtiles["wd0s"][:, wc],
                             h_cur[:, L - d:L], start=False, stop=True)

        # a = relu(psum + b_dil) on DVE; capture last col for skip path
        a_sb = work.tile([128, L], F32R, tag="a")
        nc.vector.tensor_scalar(a_sb[:], ps_a[:], btiles["bd"][:, i:i + 1],
                                0.0, op0=ALU.add, op1=ALU.max)
        nc.vector.tensor_scalar(s_sb[:, i:i + 1], ps_a[:, L - 1:L],
                                btiles["bd"][:, i:i + 1], 0.0,
                                op0=ALU.add, op1=ALU.max)

        ps_f = pspool.tile([128, L], F32, tag="ps")
        ps_g = pspool.tile([128, L], F32, tag="ps")
        nc.tensor.matmul(ps_f[:], wtiles["wf"][:, wc], a_sb[:],
                         start=True, stop=True)
        nc.tensor.matmul(ps_g[:], wtiles["wg"][:, wc], a_sb[:],
                         start=True, stop=True)

        f_sb = work.tile([128, L], F32, tag="f")
        nc.scalar.activation(f_sb[:], ps_f[:], AF.Tanh,
                             bias=btiles["bf"][:, i:i + 1])
        g_sb = work.tile([128, L], F32, tag="g")
        nc.scalar.activation(g_sb[:], ps_g[:], AF.Sigmoid,
                             bias=btiles["bg"][:, i:i + 1])
        fg_sb = work.tile([128, L], F32R, tag="fg")
        nc.vector.tensor_mul(fg_sb[:], f_sb[:], g_sb[:])

        ps_o = pspool.tile([128, L], F32, tag="ps")
        nc.tensor.matmul(ps_o[:], wtiles["wo"][:, wc], fg_sb[:],
                         start=True, stop=True)

        # h_new = psum_o + h_old + b_out
        h_new = hpool.tile([128, L], F32R, tag="h")
        nc.vector.tensor_tensor(h_new[:], ps_o[:], h_cur[:], op=ALU.add)
        nc.vector.tensor_scalar_add(h_new[:], h_new[:], btiles["bo"][:, i:i + 1])
        h_cur = h_new

    # ---- skip head: PE-transpose S [128, NB] -> [NB, 128], then contract
    # over blocks i with per-channel strided-free matmuls.
    ps_t = pspool1.tile([NB, 128], F32R, tag="pst")
    nc.tensor.transpose(ps_t[:], s_sb[:], ident_sb[:])
    st_sb = work.tile([NB, 128], F32R, tag="st")
    nc.scalar.copy(st_sb[:], ps_t[:])

    y_parts = []
    s1 = []
    ps_sk = [pspool.tile([128, BL], F32, tag="ps", name=f"ps_sk{_m}")
             for _m in range(2)]
    for mh in range(2):
        for c in range(C):
            wv = wsk_sb[:, (c * 2 + mh) * 128:(c * 2 + mh + 1) * 128]
            # rhs: st_sb[i, 24 + c + 32*b] for b in 0..3 (free stride 32)
            rv = st_sb[:, 24 + c::32]
            nc.tensor.matmul(ps_sk[mh][:], wv, rv,
                             start=(c == 0), stop=(c == C - 1))
    for mh in range(2):
        t = work.tile([128, BL], F32R, tag=f"s1_{mh}")
        nc.scalar.activation(t[:], ps_sk[mh][:], AF.Relu,
                             bias=btiles["bsk"][:, mh:mh + 1])
        s1.append(t)
    s2 = []
    for mh in range(2):
        ps1 = pspool.tile([128, BL], F32, tag="ps")
        for kh in range(2):
            wv = wo1_sb[:, (kh * 2 + mh) * 128:(kh * 2 + mh + 1) * 128]
            nc.tensor.matmul(ps1[:], wv, s1[kh][:],
                             start=(kh == 0), stop=(kh == 1))
        t = work.tile([128, BL], F32R, tag=f"s2_{mh}")
        nc.scalar.activation(t[:], ps1[:], AF.Relu,
                             bias=btiles["bo1"][:, mh:mh + 1])
        s2.append(t)
    for mh in range(2):
        ps2 = pspool.tile([128, BL], F32, tag="ps")
        for kh in range(2):
            wv = wo2_sb[:, (kh * 2 + mh) * 128:(kh * 2 + mh + 1) * 128]
            nc.tensor.matmul(ps2[:], wv, s2[kh][:],
                             start=(kh == 0), stop=(kh == 1))
        t = work.tile([128, BL], F32, tag=f"y{mh}")
        nc.scalar.activation(t[:], ps2[:], AF.Identity,
                             bias=btiles["bo2"][:, mh:mh + 1])
        y_parts.append(t)

    out_view = out_d.rearrange("b (mh o) -> mh o b", mh=2)
    for mh in range(2):
        nc.sync.dma_start(out_view[mh], y_parts[mh][:])

    ctx.close()


# ------------------------------------------------- wait-split fixup
# walrus in this toolchain rejects instructions with >2 sync waits; hoist
# excess waits onto no-ops inserted just before the offending instruction.


def _split_excess_waits(nc, cap=2):
    counter = [0]
    for fn in nc.m.functions:
        for blk in fn.blocks:
            insts = blk.instructions
            out = []
            changed = False
            for inst in insts:
                cap = 1
                si = inst.sync_info
                waits = list(si.on_wait) if si is not None else []
                if len(waits) > cap:
                    changed = True
                    extra, keep = waits[:-cap], waits[-cap:]
                    for j in range(0, len(extra), cap):
                        grp = extra[j:j + cap]
                        nop = mybir.InstNoOp(
                            name=f"wait-split-{counter[0]}", ins=[], outs=[])
                        counter[0] += 1
                        nop.engine = inst.engine
                        nop.sync_info = mybir.SyncInfo(on_wait=grp,
                                                       on_update=[])
                        out.append(nop)
                    inst.sync_info = mybir.SyncInfo(
                        on_wait=keep, on_update=list(si.on_update))
                out.append(inst)
            if changed:
                blk.instructions = out


# ------------------------------------------------- host-side packing


def _pack_params(p):
    """Build the device-layout parameter arrays (replicated per core)."""
    f32 = np.float32
    w_in = np.asarray(p["w_in"], f32)       # (8, 1)
    b_in = np.asarray(p["b_in"], f32)       # (8,)
    w_dil = np.asarray(p["w_dil"], f32)     # (NB, 8, 8, 2)
    b_dil = np.asarray(p["b_dil"], f32)     # (NB, 8)
    w_skip = np.asarray(p["w_skip"], f32)   # (NB, SK, 8)
    b_skip = np.asarray(p["b_skip"], f32)   # (NB, SK)
    w_f = np.asarray(p["w_f"], f32)
    b_f = np.asarray(p["b_f"], f32)
    w_g = np.asarray(p["w_g"], f32)
    b_g = np.asarray(p["b_g"], f32)
    w_out = np.asarray(p["w_out"], f32)
    b_out = np.asarray(p["b_out"], f32)
    w_o1 = np.asarray(p["w_o1"], f32)       # (SK, SK)
    b_o1 = np.asarray(p["b_o1"], f32)
    w_o2 = np.asarray(p["w_o2"], f32)
    b_o2 = np.asarray(p["b_o2"], f32)

    cvec = np.arange(128) % C               # channel per partition

    def strip_pack(mat_per_block, shifted=False):
        """mat_per_block: (NB, 8out, 8in) -> [128, NB*128] block-diag lhsT."""
        out = np.zeros((128, NB * 128), f32)
        for i in range(NB):
            for j in range(16):
                if shifted:
                    if j % 4 == 0:
                        continue
                    krow = 8 * (j - 1)
                else:
                    krow = 8 * j
                mcol = 128 * i + 8 * j
                # lhsT[k=in_c, m=out_o] = W[o, c]
                out[krow:krow + 8, mcol:mcol + 8] = mat_per_block[i].T
        return out

    wd1_h = strip_pack(w_dil[:, :, :, 1])
    wd0_h = strip_pack(w_dil[:, :, :, 0])
    wd0s_h = strip_pack(w_dil[:, :, :, 0], shifted=True)
    wf_h = strip_pack(w_f)
    wg_h = strip_pack(w_g)
    wo_h = strip_pack(w_out)

    win_h = np.zeros((BL * TCH, 128), f32)
    for j in range(BL * TCH):
        win_h[j, 8 * j:8 * j + 8] = w_in[:, 0]

    bin_h = b_in[cvec][:, None].astype(f32)
    bd_h = b_dil.T[cvec, :]                 # [128, NB]
    bf_h = b_f.T[cvec, :]
    bg_h = b_g.T[cvec, :]
    bo_h = b_out.T[cvec, :]

    # skip weights for transpose-contract: lhsT[c,mh][i, skm] = w_skip[i, 128mh+skm, c]
    wsk_h = np.zeros((NB, 16 * 128), f32)
    for c in range(C):
        for mh in range(2):
            wsk_h[:, (c * 2 + mh) * 128:(c * 2 + mh + 1) * 128] = \
                w_skip[:, 128 * mh:128 * (mh + 1), c]

    wo1_h4 = np.zeros((2, 2, 128, 128), f32)
    wo2_h4 = np.zeros((2, 2, 128, 128), f32)
    for kh in range(2):
        for mh in range(2):
            wo1_h4[kh, mh] = w_o1[128 * mh:128 * (mh + 1),
                                  128 * kh:128 * (kh + 1)].T
            wo2_h4[kh, mh] = w_o2[128 * mh:128 * (mh + 1),
                                  128 * kh:128 * (kh + 1)].T

    bsk_h = np.stack([b_skip.sum(0)[:128], b_skip.sum(0)[128:]], axis=1)
    bo1_h = np.stack([b_o1[:128], b_o1[128:]], axis=1)
    bo2_h = np.stack([b_o2[:128], b_o2[128:]], axis=1)

    # flatten [kh][mh] 128x128 chunks into [128, 4*128] (col-block order kh*2+mh)
    def flat4(w):
        return np.concatenate([w[kh, mh] for kh in range(2) for mh in range(2)],
                              axis=1)

    wo1_h = flat4(wo1_h4)
    wo2_h = flat4(wo2_h4)
    ident_h = np.eye(128, dtype=f32)

    return dict(win=win_h, wd1=wd1_h, wd0=wd0_h, wd0s=wd0s_h, wf=wf_h,
                wg=wg_h, wo=wo_h, bin=bin_h, bd=bd_h, bf=bf_h, bg=bg_h,
                bo=bo_h, wsk=wsk_h, ident=ident_h, bsk=bsk_h, wo1=wo1_h,
                bo1=bo1_h, wo2=wo2_h, bo2=bo2_h)


_NC_CACHE = {}


def get_nc(fixup=True):
    key = "nc" if fixup else "nc_nofix"
    if key not in _NC_CACHE:
        _NC_CACHE[key] = _build_nc(fixup=fixup)
    return _NC_CACHE[key]


def make_in_maps(**inputs):
    x = np.asarray(inputs["x"], np.float32)
    params = _pack_params(inputs)
    in_maps = []
    for k in range(N_CORES):
        shard = x[BL * k:BL * (k + 1)]                       # (4, 2048)
        x_l = shard.reshape(BL, TCH, L).reshape(BL * TCH, L).copy()
        m = {"x": x_l}
        m.update(params)
        in_maps.append(m)
    return in_maps


def kernel(**inputs):
    nc = get_nc()
    in_maps = make_in_maps(**inputs)
    res = run_bass_kernel_spmd(nc, in_maps, list(range(N_CORES)))
    outs = [res.results[k]["out"] for k in range(N_CORES)]
    return np.concatenate(outs, axis=0).astype(np.float32)


# revision 31
# speedup vs baseline: 1.1980x; 1.1980x over previous
"""WaveNet-like dense CNN on 8 TRN2 NeuronCores — batch data parallel.

Self-contained: hardcodes shapes from the problem spec.
  x: (32, 2048) f32 -> out: (32, 256) f32
Layout per core (4 batch samples): partitions p = b*32 + tc*8 + c
(b in 0..3 sample, tc in 0..3 time-chunk of 512, c in 0..7 channel),
free axis = t_low in 0..511.  All 1x1 convs are block-diagonal 64x64
strip matmuls (2 concurrent tile_position strips).  The dilated causal
conv is 3 matmuls (full tap1, shifted-view tap0, cross-chunk boundary
tap0 with a partition-shifting weight) — no data movement for shifts.
Only skip[:, :, -1] is ever used, so skip convs run on 1 column/block.
"""

import sys

sys.path.insert(0, "/opt/trn_rl_repo")

import numpy as np

import concourse.bass as bass
import concourse.tile as tile
from concourse import mybir
from concourse.bass_utils import run_bass_kernel_spmd

F32 = mybir.dt.float32
F32R = mybir.dt.float32r
AF = mybir.ActivationFunctionType
ALU = mybir.AluOpType

SEQ_LEN = 2048
C = 8
SK = 256
NB = 32
B = 32
N_CORES = 8
BL = B // N_CORES      # 4 samples per core
TCH = 4                # time chunks per sample
L = SEQ_LEN // TCH     # 512 free elems
NSTRIP = 2             # two 64x64 tile_position strips
SW = 64                # strip width (partitions/outputs per strip)
GPS = 8                # groups per strip (8 groups of 8 channels)


def _dil(i):
    return 2 ** (i % 8)


# ---------------------------------------------------------------- build


def _build_nc(fixup=True):
    nc = bass.Bass("TRN2", target_bir_lowering=False, debug=False,
                   num_devices=N_CORES)

    def din(name, shape, dt=F32R):
        return nc.dram_tensor(name, shape, dt, kind="ExternalInput").ap()

    x_d = din("x", [BL * TCH, L])
    win_d = din("win", [BL * TCH, 128])
    # [NB, 128, 128]: contiguous per block so the per-block DMA reads
    # DRAM sequentially (full burst bandwidth)
    wd1_d = din("wd1", [NB, 128, 128])
    wd0_d = din("wd0", [NB, 128, 128])
    wd0s_d = din("wd0s", [NB, 128, 128])
    wf_d = din("wf", [NB, 128, 128])
    wg_d = din("wg", [NB, 128, 128])
    wo_d = din("wo", [NB, 128, 128])
    bin_d = din("bin", [128, 1], F32)
    bd_d = din("bd", [128, NB], F32)
    bf_d = din("bf", [128, NB], F32)
    bg_d = din("bg", [128, NB], F32)
    bo_d = din("bo", [128, NB], F32)
    wsk_d = din("wsk", [32, 16 * 128])
    ident_d = din("ident", [128, 128])
    bsk_d = din("bsk", [128, 2], F32)
    wo1_d = din("wo1", [128, 4 * 128])
    bo1_d = din("bo1", [128, 2], F32)
    wo2_d = din("wo2", [128, 4 * 128])
    bo2_d = din("bo2", [128, 2], F32)

    out_d = nc.dram_tensor("out", [BL, SK], F32, kind="ExternalOutput").ap()

    with tile.TileContext(nc) as tc:
        _emit(nc, tc, x_d, win_d, wd1_d, wd0_d, wd0s_d, wf_d, wg_d, wo_d,
              bin_d, bd_d, bf_d, bg_d, bo_d, wsk_d, ident_d, bsk_d,
              wo1_d, bo1_d, wo2_d, bo2_d, out_d)

    if fixup:
        _split_excess_waits(nc)
    return nc


def _emit(nc, tc, x_d, win_d, wd1_d, wd0_d, wd0s_d, wf_d, wg_d, wo_d,
          bin_d, bd_d, bf_d, bg_d, bo_d, wsk_d, ident_d, bsk_d,
          wo1_d, bo1_d, wo2_d, bo2_d, out_d):
    from contextlib import ExitStack
    ctx = ExitStack()
    const = ctx.enter_context(tc.tile_pool(name="const", bufs=1))
    work = ctx.enter_context(tc.tile_pool(name="work", bufs=2))
    hpool = ctx.enter_context(tc.tile_pool(name="h", bufs=3))
    pspool = ctx.enter_context(tc.tile_pool(name="ps", bufs=6, space="PSUM"))
    pspool1 = ctx.enter_context(tc.tile_pool(name="ps1", bufs=1, space="PSUM"))

    # ---- persistent loads (small params first so compute starts early;
    # bulk per-block weights stream in behind and hide under compute)
    x_sb = const.tile([BL * TCH, L], F32R, tag="x")
    nc.sync.dma_start(x_sb[:], x_d[:])
    win_sb = const.tile([BL * TCH, 128], F32R, tag="win")
    nc.sync.dma_start(win_sb[:], win_d[:])

    btiles = {}
    for nm, d, w in (("bin", bin_d, 1), ("bd", bd_d, NB), ("bf", bf_d, NB),
                     ("bg", bg_d, NB), ("bo", bo_d, NB), ("bsk", bsk_d, 2),
                     ("bo1", bo1_d, 2), ("bo2", bo2_d, 2)):
        t = const.tile([128, w], F32, tag=nm)
        nc.sync.dma_start(t[:], d[:])
        btiles[nm] = t

    wsk_sb = const.tile([32, 16 * 128], F32R, tag="wsk")
    nc.sync.dma_start(wsk_sb[:], wsk_d[:])
    ident_sb = const.tile([128, 128], F32R, tag="ident")
    nc.sync.dma_start(ident_sb[:], ident_d[:])
    wo1_sb = const.tile([128, 4 * 128], F32R, tag="wo1")
    nc.sync.dma_start(wo1_sb[:], wo1_d[:])
    wo2_sb = const.tile([128, 4 * 128], F32R, tag="wo2")
    nc.sync.dma_start(wo2_sb[:], wo2_d[:])

    wtiles = {}
    wnames = (("wd1", wd1_d), ("wd0", wd0_d), ("wd0s", wd0s_d),
              ("wf", wf_d), ("wg", wg_d), ("wo", wo_d))
    for nm, _ in wnames:
        t = const.tile([128, NB * 128], F32R, tag=nm)
        wtiles[nm] = t
    # per-block DMA order: all six weights of block 0 first, then block 1...
    for i in range(NB):
        for nm, d in wnames:
            nc.sync.dma_start(wtiles[nm][:, 128 * i:128 * (i + 1)], d[i])

    s_sb = const.tile([128, NB], F32R, tag="scap")

    # ---- input 1x1 conv: h0 = w_in * x + b_in  (K=16 matmul broadcast)
    ps_h = pspool.tile([128, L], F32, tag="ps")
    nc.tensor.matmul(ps_h[:], win_sb[:], x_sb[:], start=True, stop=True)
    h_cur = hpool.tile([128, L], F32R, tag="h")
    nc.scalar.activation(h_cur[:], ps_h[:], AF.Identity,
                         bias=btiles["bin"][:, 0:1])

    # ---- 32 residual blocks
    for i in range(NB):
        d = _dil(i)
        wc = slice(128 * i, 128 * (i + 1))

        ps_a = pspool.tile([128, L], F32, tag="ps")
        # tap1 (current sample) over all columns
        nc.tensor.matmul(ps_a[:], wtiles["wd1"][:, wc], h_cur[:],
                         start=True, stop=False)
        if d == 1:
            # fp32r MMs need even column counts and 8B-aligned column
            # starts, so the d=1 case gets even-sized MMs plus 2 fixups.
            nc.tensor.matmul(ps_a[:, 2:L], wtiles["wd0"][:, wc],
                             h_cur[:, 1:L - 1], start=False, stop=True)
            ps_bc = pspool.tile([128, 4], F32, tag="ps")
            nc.tensor.matmul(ps_bc[:, 0:2], wtiles["wd0s"][:, wc],
                             h_cur[:, L - 2:L], start=True, stop=True,
                             skip_group_check=True)
            nc.tensor.matmul(ps_bc[:, 2:4], wtiles["wd0"][:, wc],
                             h_cur[:, 0:2], start=True, stop=True,
                             skip_group_check=True)
            # col0 += shifted-tap(prev chunk last col); col1 += tap0(h[0])
            fix_sb = work.tile([128, 2], F32, tag="fix")
            nc.scalar.copy(fix_sb[:, 0:1], ps_bc[:, 1:2])
            nc.scalar.copy(fix_sb[:, 1:2], ps_bc[:, 2:3])
            nc.vector.tensor_tensor(ps_a[:, 0:2], ps_a[:, 0:2],
                                    fix_sb[:], op=ALU.add)
        else:
            # tap0 intra-chunk: out cols [d:512] <- h cols [0:512-d]
            nc.tensor.matmul(ps_a[:, d:L], wtiles["wd0"][:, wc],
                             h_cur[:, 0:L - d], start=False, stop=False)
            # tap0 cross-chunk boundary: out cols [0:d] <- prev chunk tail
            # (weight maps group tc-1 -> tc; tc=0 groups get zero pad)
            nc.tensor.matmul(ps_a[:, 0:d], wtiles["wd0s"][:, wc],
                             h_cur[:, L - d:L], start=False, stop=True)

        # a = relu(psum + b_dil) on DVE; capture last col for skip path
        a_sb = work.tile([128, L], F32R, tag="a")
        nc.vector.tensor_scalar(a_sb[:], ps_a[:], btiles["bd"][:, i:i + 1],
                                0.0, op0=ALU.add, op1=ALU.max)
        nc.vector.tensor_scalar(s_sb[:, i:i + 1], ps_a[:, L - 1:L],
                                btiles["bd"][:, i:i + 1], 0.0,
                                op0=ALU.add, op1=ALU.max)

        ps_f = pspool.tile([128, L], F32, tag="ps")
        ps_g = pspool.tile([128, L], F32, tag="ps")
        nc.tensor.matmul(ps_f[:], wtiles["wf"][:, wc], a_sb[:],
                         start=True, stop=True)
        nc.tensor.matmul(ps_g[:], wtiles["wg"][:, wc], a_sb[:],
                         start=True, stop=True)

        f_sb = work.tile([128, L], F32, tag="f")
        nc.scalar.activation(f_sb[:], ps_f[:], AF.Tanh,
                             bias=btiles["bf"][:, i:i + 1])
        g_sb = work.tile([128, L], F32, tag="g")
        nc.scalar.activation(g_sb[:], ps_g[:], AF.Sigmoid,
                             bias=btiles["bg"][:, i:i + 1])
        fg_sb = work.tile([128, L], F32R, tag="fg")
        nc.vector.tensor_mul(fg_sb[:], f_sb[:], g_sb[:])

        ps_o = pspool.tile([128, L], F32, tag="ps")
        nc.tensor.matmul(ps_o[:], wtiles["wo"][:, wc], fg_sb[:],
                         start=True, stop=True)

        # h_new = psum_o + h_old + b_out
        h_new = hpool.tile([128, L], F32R, tag="h")
        nc.vector.tensor_tensor(h_new[:], ps_o[:], h_cur[:], op=ALU.add)
        nc.vector.tensor_scalar_add(h_new[:], h_new[:], btiles["bo"][:, i:i + 1])
        h_cur = h_new

    # ---- skip head: PE-transpose S [128, NB] -> [NB, 128], then contract
    # over blocks i with per-channel strided-free matmuls.
    ps_t = pspool1.tile([NB, 128], F32R, tag="pst")
    nc.tensor.transpose(ps_t[:], s_sb[:], ident_sb[:])
    st_sb = work.tile([NB, 128], F32R, tag="st")
    nc.scalar.copy(st_sb[:], ps_t[:])

    y_parts = []
    s1 = []
    ps_sk = [pspool.tile([128, BL], F32, tag="ps", name=f"ps_sk{_m}")
             for _m in range(2)]
    for mh in range(2):
        for c in range(C):
            wv = wsk_sb[:, (c * 2 + mh) * 128:(c * 2 + mh + 1) * 128]
            # rhs: st_sb[i, 24 + c + 32*b] for b in 0..3 (free stride 32)
            rv = st_sb[:, 24 + c::32]
            nc.tensor.matmul(ps_sk[mh][:], wv, rv,
                             start=(c == 0), stop=(c == C - 1))
    for mh in range(2):
        t = work.tile([128, BL], F32R, tag=f"s1_{mh}")
        nc.scalar.activation(t[:], ps_sk[mh][:], AF.Relu,
                             bias=btiles["bsk"][:, mh:mh + 1])
        s1.append(t)
    s2 = []
    for mh in range(2):
        ps1 = pspool.tile([128, BL], F32, tag="ps")
        for kh in range(2):
            wv = wo1_sb[:, (kh * 2 + mh) * 128:(kh * 2 + mh + 1) * 128]
            nc.tensor.matmul(ps1[:], wv, s1[kh][:],
                             start=(kh == 0), stop=(kh == 1))
        t = work.tile([128, BL], F32R, tag=f"s2_{mh}")
        nc.scalar.activation(t[:], ps1[:], AF.Relu,
                             bias=btiles["bo1"][:, mh:mh + 1])
        s2.append(t)
    for mh in range(2):
        ps2 = pspool.tile([128, BL], F32, tag="ps")
        for kh in range(2):
            wv = wo2_sb[:, (kh * 2 + mh) * 128:(kh * 2 + mh + 1) * 128]
            nc.tensor.matmul(ps2[:], wv, s2[kh][:],
                             start=(kh == 0), stop=(kh == 1))
        t = work.tile([128, BL], F32, tag=f"y{mh}")
        nc.scalar.activation(t[:], ps2[:], AF.Identity,
                             bias=btiles["bo2"][:, mh:mh + 1])
        y_parts.append(t)

    out_view = out_d.rearrange("b (mh o) -> mh o b", mh=2)
    for mh in range(2):
        nc.sync.dma_start(out_view[mh], y_parts[mh][:])

    ctx.close()


# ------------------------------------------------- wait-split fixup
# walrus in this toolchain rejects instructions with >2 sync waits; hoist
# excess waits onto no-ops inserted just before the offending instruction.


def _split_excess_waits(nc, cap=2):
    counter = [0]
    for fn in nc.m.functions:
        for blk in fn.blocks:
            insts = blk.instructions
            out = []
            changed = False
            for inst in insts:
                cap = 1
                si = inst.sync_info
                waits = list(si.on_wait) if si is not None else []
                if len(waits) > cap:
                    changed = True
                    extra, keep = waits[:-cap], waits[-cap:]
                    for j in range(0, len(extra), cap):
                        grp = extra[j:j + cap]
                        nop = mybir.InstNoOp(
                            name=f"wait-split-{counter[0]}", ins=[], outs=[])
                        counter[0] += 1
                        nop.engine = inst.engine
                        nop.sync_info = mybir.SyncInfo(on_wait=grp,
                                                       on_update=[])
                        out.append(nop)
                    inst.sync_info = mybir.SyncInfo(
                        on_wait=keep, on_update=list(si.on_update))
                out.append(inst)
            if changed:
                blk.instructions = out


# ------------------------------------------------- host-side packing


def _pack_params(p):
    """Build the device-layout parameter arrays (replicated per core)."""
    f32 = np.float32
    w_in = np.asarray(p["w_in"], f32)       # (8, 1)
    b_in = np.asarray(p["b_in"], f32)       # (8,)
    w_dil = np.asarray(p["w_dil"], f32)     # (NB, 8, 8, 2)
    b_dil = np.asarray(p["b_dil"], f32)     # (NB, 8)
    w_skip = np.asarray(p["w_skip"], f32)   # (NB, SK, 8)
    b_skip = np.asarray(p["b_skip"], f32)   # (NB, SK)
    w_f = np.asarray(p["w_f"], f32)
    b_f = np.asarray(p["b_f"], f32)
    w_g = np.asarray(p["w_g"], f32)
    b_g = np.asarray(p["b_g"], f32)
    w_out = np.asarray(p["w_out"], f32)
    b_out = np.asarray(p["b_out"], f32)
    w_o1 = np.asarray(p["w_o1"], f32)       # (SK, SK)
    b_o1 = np.asarray(p["b_o1"], f32)
    w_o2 = np.asarray(p["w_o2"], f32)
    b_o2 = np.asarray(p["b_o2"], f32)

    cvec = np.arange(128) % C               # channel per partition

    def strip_pack(mat_per_block, shifted=False):
        """mat_per_block: (NB, 8out, 8in) -> [128, NB*128] block-diag lhsT."""
        out = np.zeros((128, NB * 128), f32)
        for i in range(NB):
            for j in range(16):
                if shifted:
                    if j % 4 == 0:
                        continue
                    krow = 8 * (j - 1)
                else:
                    krow = 8 * j
                mcol = 128 * i + 8 * j
                # lhsT[k=in_c, m=out_o] = W[o, c]
                out[krow:krow + 8, mcol:mcol + 8] = mat_per_block[i].T
        return out

    def to_blocks(w):
        # [128, NB*128] -> [NB, 128, 128] contiguous per block
        return np.ascontiguousarray(
            w.reshape(128, NB, 128).transpose(1, 0, 2))

    wd1_h = to_blocks(strip_pack(w_dil[:, :, :, 1]))
    wd0_h = to_blocks(strip_pack(w_dil[:, :, :, 0]))
    wd0s_h = to_blocks(strip_pack(w_dil[:, :, :, 0], shifted=True))
    wf_h = to_blocks(strip_pack(w_f))
    wg_h = to_blocks(strip_pack(w_g))
    wo_h = to_blocks(strip_pack(w_out))

    win_h = np.zeros((BL * TCH, 128), f32)
    for j in range(BL * TCH):
        win_h[j, 8 * j:8 * j + 8] = w_in[:, 0]

    bin_h = b_in[cvec][:, None].astype(f32)
    bd_h = b_dil.T[cvec, :]                 # [128, NB]
    bf_h = b_f.T[cvec, :]
    bg_h = b_g.T[cvec, :]
    bo_h = b_out.T[cvec, :]

    # skip weights for transpose-contract: lhsT[c,mh][i, skm] = w_skip[i, 128mh+skm, c]
    wsk_h = np.zeros((NB, 16 * 128), f32)
    for c in range(C):
        for mh in range(2):
            wsk_h[:, (c * 2 + mh) * 128:(c * 2 + mh + 1) * 128] = \
                w_skip[:, 128 * mh:128 * (mh + 1), c]

    wo1_h4 = np.zeros((2, 2, 128, 128), f32)
    wo2_h4 = np.zeros((2, 2, 128, 128), f32)
    for kh in range(2):
        for mh in range(2):
            wo1_h4[kh, mh] = w_o1[128 * mh:128 * (mh + 1),
                                  128 * kh:128 * (kh + 1)].T
            wo2_h4[kh, mh] = w_o2[128 * mh:128 * (mh + 1),
                                  128 * kh:128 * (kh + 1)].T

    bsk_h = np.stack([b_skip.sum(0)[:128], b_skip.sum(0)[128:]], axis=1)
    bo1_h = np.stack([b_o1[:128], b_o1[128:]], axis=1)
    bo2_h = np.stack([b_o2[:128], b_o2[128:]], axis=1)

    # flatten [kh][mh] 128x128 chunks into [128, 4*128] (col-block order kh*2+mh)
    def flat4(w):
        return np.concatenate([w[kh, mh] for kh in range(2) for mh in range(2)],
                              axis=1)

    wo1_h = flat4(wo1_h4)
    wo2_h = flat4(wo2_h4)
    ident_h = np.eye(128, dtype=f32)

    return dict(win=win_h, wd1=wd1_h, wd0=wd0_h, wd0s=wd0s_h, wf=wf_h,
                wg=wg_h, wo=wo_h, bin=bin_h, bd=bd_h, bf=bf_h, bg=bg_h,
                bo=bo_h, wsk=wsk_h, ident=ident_h, bsk=bsk_h, wo1=wo1_h,
                bo1=bo1_h, wo2=wo2_h, bo2=bo2_h)


_NC_CACHE = {}


def get_nc(fixup=True):
    key = "nc" if fixup else "nc_nofix"
    if key not in _NC_CACHE:
        _NC_CACHE[key] = _build_nc(fixup=fixup)
    return _NC_CACHE[key]


def make_in_maps(**inputs):
    x = np.asarray(inputs["x"], np.float32)
    params = _pack_params(inputs)
    in_maps = []
    for k in range(N_CORES):
        shard = x[BL * k:BL * (k + 1)]                       # (4, 2048)
        x_l = shard.reshape(BL, TCH, L).reshape(BL * TCH, L).copy()
        m = {"x": x_l}
        m.update(params)
        in_maps.append(m)
    return in_maps


def kernel(**inputs):
    nc = get_nc()
    in_maps = make_in_maps(**inputs)
    res = run_bass_kernel_spmd(nc, in_maps, list(range(N_CORES)))
    outs = [res.results[k]["out"] for k in range(N_CORES)]
    return np.concatenate(outs, axis=0).astype(np.float32)


# revision 33
# speedup vs baseline: 1.5978x; 1.3338x over previous
"""WaveNet-like dense CNN on 8 TRN2 NeuronCores — batch data parallel.

Self-contained: hardcodes shapes from the problem spec.
  x: (32, 2048) f32 -> out: (32, 256) f32
Layout per core (4 batch samples): partitions p = b*32 + tc*8 + c
(b in 0..3 sample, tc in 0..3 time-chunk of 512, c in 0..7 channel),
free axis = t_low in 0..511.  All 1x1 convs are block-diagonal 64x64
strip matmuls (2 concurrent tile_position strips).  The dilated causal
conv is 3 matmuls (full tap1, shifted-view tap0, cross-chunk boundary
tap0 with a partition-shifting weight) — no data movement for shifts.
Only skip[:, :, -1] is ever used, so skip convs run on 1 column/block.
"""

import sys

sys.path.insert(0, "/opt/trn_rl_repo")

import numpy as np

import concourse.bass as bass
import concourse.tile as tile
from concourse import mybir
from concourse.bass_utils import run_bass_kernel_spmd

F32 = mybir.dt.float32
F32R = mybir.dt.float32r
AF = mybir.ActivationFunctionType
ALU = mybir.AluOpType

SEQ_LEN = 2048
C = 8
SK = 256
NB = 32
B = 32
N_CORES = 8
BL = B // N_CORES      # 4 samples per core
TCH = 4                # time chunks per sample
L = SEQ_LEN // TCH     # 512 free elems
NSTRIP = 2             # two 64x64 tile_position strips
SW = 64                # strip width (partitions/outputs per strip)
GPS = 8                # groups per strip (8 groups of 8 channels)


def _dil(i):
    return 2 ** (i % 8)


# ---------------------------------------------------------------- build


def _build_nc(fixup=True):
    nc = bass.Bass("TRN2", target_bir_lowering=False, debug=False,
                   num_devices=N_CORES)

    def din(name, shape, dt=F32R):
        return nc.dram_tensor(name, shape, dt, kind="ExternalInput").ap()

    x_d = din("x", [BL * TCH, L])
    win_d = din("win", [BL * TCH, 128])
    # [NB, 128, 128]: contiguous per block so the per-block DMA reads
    # DRAM sequentially (full burst bandwidth)
    wd1_d = din("wd1", [NB, 128, 128])
    wd0_d = din("wd0", [NB, 128, 128])
    wd0s_d = din("wd0s", [NB, 128, 128])
    wf_d = din("wf", [NB, 128, 128])
    wg_d = din("wg", [NB, 128, 128])
    wo_d = din("wo", [NB, 128, 128])
    bin_d = din("bin", [128, 1], F32)
    bd_d = din("bd", [128, NB], F32)
    bf_d = din("bf", [128, NB], F32)
    bg_d = din("bg", [128, NB], F32)
    bo_d = din("bo", [128, NB], F32)
    wsk_d = din("wsk", [32, 16 * 128])
    ident_d = din("ident", [128, 128])
    bsk_d = din("bsk", [128, 2], F32)
    wo1_d = din("wo1", [128, 4 * 128])
    bo1_d = din("bo1", [128, 2], F32)
    wo2_d = din("wo2", [128, 4 * 128])
    bo2_d = din("bo2", [128, 2], F32)

    out_d = nc.dram_tensor("out", [BL, SK], F32, kind="ExternalOutput").ap()

    with tile.TileContext(nc) as tc:
        _emit(nc, tc, x_d, win_d, wd1_d, wd0_d, wd0s_d, wf_d, wg_d, wo_d,
              bin_d, bd_d, bf_d, bg_d, bo_d, wsk_d, ident_d, bsk_d,
              wo1_d, bo1_d, wo2_d, bo2_d, out_d)

    if fixup:
        _split_excess_waits(nc)
    return nc


def _emit(nc, tc, x_d, win_d, wd1_d, wd0_d, wd0s_d, wf_d, wg_d, wo_d,
          bin_d, bd_d, bf_d, bg_d, bo_d, wsk_d, ident_d, bsk_d,
          wo1_d, bo1_d, wo2_d, bo2_d, out_d):
    from contextlib import ExitStack
    ctx = ExitStack()
    const = ctx.enter_context(tc.tile_pool(name="const", bufs=1))
    work = ctx.enter_context(tc.tile_pool(name="work", bufs=2))
    hpool = ctx.enter_context(tc.tile_pool(name="h", bufs=3))
    pspool = ctx.enter_context(tc.tile_pool(name="ps", bufs=3, space="PSUM"))
    pspool_f = ctx.enter_context(tc.tile_pool(name="psf", bufs=1, space="PSUM"))
    pspool_g = ctx.enter_context(tc.tile_pool(name="psg", bufs=1, space="PSUM"))
    pspool_o = ctx.enter_context(tc.tile_pool(name="pso", bufs=2, space="PSUM"))
    pspool1 = ctx.enter_context(tc.tile_pool(name="ps1", bufs=1, space="PSUM"))

    # ---- persistent loads (small params first so compute starts early;
    # bulk per-block weights stream in behind and hide under compute)
    x_sb = const.tile([BL * TCH, L], F32R, tag="x")
    nc.sync.dma_start(x_sb[:], x_d[:])
    win_sb = const.tile([BL * TCH, 128], F32R, tag="win")
    nc.sync.dma_start(win_sb[:], win_d[:])

    btiles = {}
    for nm, d, w in (("bin", bin_d, 1), ("bd", bd_d, NB), ("bf", bf_d, NB),
                     ("bg", bg_d, NB), ("bo", bo_d, NB), ("bsk", bsk_d, 2),
                     ("bo1", bo1_d, 2), ("bo2", bo2_d, 2)):
        t = const.tile([128, w], F32, tag=nm)
        nc.sync.dma_start(t[:], d[:])
        btiles[nm] = t

    wsk_sb = const.tile([32, 16 * 128], F32R, tag="wsk")
    nc.sync.dma_start(wsk_sb[:], wsk_d[:])
    ident_sb = const.tile([128, 128], F32R, tag="ident")
    nc.sync.dma_start(ident_sb[:], ident_d[:])
    wo1_sb = const.tile([128, 4 * 128], F32R, tag="wo1")
    nc.sync.dma_start(wo1_sb[:], wo1_d[:])
    wo2_sb = const.tile([128, 4 * 128], F32R, tag="wo2")
    nc.sync.dma_start(wo2_sb[:], wo2_d[:])

    wtiles = {}
    wnames = (("wd1", wd1_d), ("wd0", wd0_d), ("wd0s", wd0s_d),
              ("wf", wf_d), ("wg", wg_d), ("wo", wo_d))
    for nm, _ in wnames:
        t = const.tile([128, NB * 128], F32R, tag=nm)
        wtiles[nm] = t
    # per-block DMA order: all six weights of block 0 first, then block 1...
    for i in range(NB):
        for nm, d in wnames:
            nc.sync.dma_start(wtiles[nm][:, 128 * i:128 * (i + 1)], d[i])

    s_sb = const.tile([128, NB], F32R, tag="scap")

    # ---- input 1x1 conv: h0 = w_in * x + b_in  (K=16 matmul broadcast)
    ps_h = pspool.tile([128, L], F32, tag="ps")
    nc.tensor.matmul(ps_h[:], win_sb[:], x_sb[:], start=True, stop=True)
    h_cur = hpool.tile([128, L], F32R, tag="h")
    nc.scalar.activation(h_cur[:], ps_h[:], AF.Identity,
                         bias=btiles["bin"][:, 0:1])

    # Receptive-field window: block i only needs output cols [c0[i], 512).
    # Only skip[:, :, -1] is used, so work shrinks down the stack.
    c0 = [0] * NB
    need = 511
    for i in reversed(range(NB)):
        c0[i] = max(0, need & ~1)
        need = max(0, c0[i] - _dil(i))

    # ---- 32 residual blocks, each split into 2 column chunks that
    # software-pipeline across engines (chunk B of block i overlaps
    # chunk A of block i, and blocks overlap each other).
    for i in range(NB):
        d = _dil(i)
        wc = slice(128 * i, 128 * (i + 1))
        o0 = c0[i]
        mid = ((o0 + L) // 2) & ~1
        chunks = [(o0, mid), (mid, L)] if mid - o0 >= 2 else [(o0, L)]
        last = i == NB - 1   # h_32 is never used: only the capture col

        ps_a = pspool.tile([128, L], F32, tag="ps")
        for (a0, a1) in chunks:
            # tap1 over [a0, a1)
            nc.tensor.matmul(ps_a[:, a0:a1], wtiles["wd1"][:, wc],
                             h_cur[:, a0:a1], start=True, stop=False,
                             skip_group_check=True)
            i0 = max(a0, 2 if d == 1 else d)
            bnd = a0 < d and d > 1
            # tap0 intra: out [i0, a1) <- h [i0-d, a1-d)
            nc.tensor.matmul(ps_a[:, i0:a1], wtiles["wd0"][:, wc],
                             h_cur[:, i0 - d:a1 - d], start=False,
                             stop=not bnd, skip_group_check=True)
            if bnd:
                # cross-chunk boundary: out [a0, d) <- prev tc-chunk tail
                nc.tensor.matmul(ps_a[:, a0:d], wtiles["wd0s"][:, wc],
                                 h_cur[:, L - d + a0:L], start=False,
                                 stop=True, skip_group_check=True)
        if d == 1 and o0 < 2:
            # fp32r MMs need even/aligned column ranges; patch cols 0,1.
            ps_bc = pspool.tile([128, 4], F32, tag="ps")
            nc.tensor.matmul(ps_bc[:, 0:2], wtiles["wd0s"][:, wc],
                             h_cur[:, L - 2:L], start=True, stop=True,
                             skip_group_check=True)
            nc.tensor.matmul(ps_bc[:, 2:4], wtiles["wd0"][:, wc],
                             h_cur[:, 0:2], start=True, stop=True,
                             skip_group_check=True)
            fix_sb = work.tile([128, 2], F32, tag="fix")
            nc.scalar.copy(fix_sb[:, 0:1], ps_bc[:, 1:2])
            nc.scalar.copy(fix_sb[:, 1:2], ps_bc[:, 2:3])
            nc.vector.tensor_tensor(ps_a[:, 0:2], ps_a[:, 0:2],
                                    fix_sb[:], op=ALU.add)

        # capture relu(a)[:, -1] for the skip head
        nc.vector.tensor_scalar(s_sb[:, i:i + 1], ps_a[:, L - 1:L],
                                btiles["bd"][:, i:i + 1], 0.0,
                                op0=ALU.add, op1=ALU.max)
        if last:
            break

        a_sb = work.tile([128, L], F32R, tag="a")
        ps_f = pspool_f.tile([128, L], F32, tag="psf")
        ps_g = pspool_g.tile([128, L], F32, tag="psg")
        ps_o = pspool_o.tile([128, L], F32, tag="pso")
        f_sb = work.tile([128, L], F32, tag="f")
        g_sb = work.tile([128, L], F32, tag="g")
        fg_sb = work.tile([128, L], F32R, tag="fg")
        h_new = hpool.tile([128, L], F32R, tag="h")
        for k, (a0, a1) in enumerate(chunks):
            cs = slice(a0, a1)
            # a = relu(psum + b_dil)
            nc.vector.tensor_scalar(a_sb[:, cs], ps_a[:, cs],
                                    btiles["bd"][:, i:i + 1], 0.0,
                                    op0=ALU.add, op1=ALU.max)
            nc.tensor.matmul(ps_f[:, cs], wtiles["wf"][:, wc], a_sb[:, cs],
                             start=True, stop=True, skip_group_check=True)
            nc.tensor.matmul(ps_g[:, cs], wtiles["wg"][:, wc], a_sb[:, cs],
                             start=True, stop=True, skip_group_check=True)
            nc.scalar.activation(f_sb[:, cs], ps_f[:, cs], AF.Tanh,
                                 bias=btiles["bf"][:, i:i + 1])
            nc.scalar.activation(g_sb[:, cs], ps_g[:, cs], AF.Sigmoid,
                                 bias=btiles["bg"][:, i:i + 1])
            nc.vector.tensor_tensor(fg_sb[:, cs], f_sb[:, cs], g_sb[:, cs],
                                    op=ALU.mult)
            # out conv + residual (via PE identity accumulate)
            nc.tensor.matmul(ps_o[:, cs], wtiles["wo"][:, wc], fg_sb[:, cs],
                             start=True, stop=False, skip_group_check=True)
            nc.tensor.matmul(ps_o[:, cs], ident_sb[:], h_cur[:, cs],
                             start=False, stop=True, skip_group_check=True)
            # h_new = ps_o + b_out (alternate engines per chunk)
            if k == 0:
                nc.scalar.activation(h_new[:, cs], ps_o[:, cs], AF.Identity,
                                     bias=btiles["bo"][:, i:i + 1])
            else:
                nc.vector.tensor_scalar_add(h_new[:, cs], ps_o[:, cs],
                                            btiles["bo"][:, i:i + 1])
        h_cur = h_new

    # ---- skip head: PE-transpose S [128, NB] -> [NB, 128], then contract
    # over blocks i with per-channel strided-free matmuls.
    ps_t = pspool1.tile([NB, 128], F32R, tag="pst")
    nc.tensor.transpose(ps_t[:], s_sb[:], ident_sb[:])
    st_sb = work.tile([NB, 128], F32R, tag="st")
    nc.scalar.copy(st_sb[:], ps_t[:])

    y_parts = []
    s1 = []
    ps_sk = [pspool.tile([128, BL], F32, tag="ps", name=f"ps_sk{_m}")
             for _m in range(2)]
    for mh in range(2):
        for c in range(C):
            wv = wsk_sb[:, (c * 2 + mh) * 128:(c * 2 + mh + 1) * 128]
            # rhs: st_sb[i, 24 + c + 32*b] for b in 0..3 (free stride 32)
            rv = st_sb[:, 24 + c::32]
            nc.tensor.matmul(ps_sk[mh][:], wv, rv,
                             start=(c == 0), stop=(c == C - 1))
    for mh in range(2):
        t = work.tile([128, BL], F32R, tag=f"s1_{mh}")
        nc.scalar.activation(t[:], ps_sk[mh][:], AF.Relu,
                             bias=btiles["bsk"][:, mh:mh + 1])
        s1.append(t)
    s2 = []
    for mh in range(2):
        ps1 = pspool.tile([128, BL], F32, tag="ps")
        for kh in range(2):
            wv = wo1_sb[:, (kh * 2 + mh) * 128:(kh * 2 + mh + 1) * 128]
            nc.tensor.matmul(ps1[:], wv, s1[kh][:],
                             start=(kh == 0), stop=(kh == 1))
        t = work.tile([128, BL], F32R, tag=f"s2_{mh}")
        nc.scalar.activation(t[:], ps1[:], AF.Relu,
                             bias=btiles["bo1"][:, mh:mh + 1])
        s2.append(t)
    for mh in range(2):
        ps2 = pspool.tile([128, BL], F32, tag="ps")
        for kh in range(2):
            wv = wo2_sb[:, (kh * 2 + mh) * 128:(kh * 2 + mh + 1) * 128]
            nc.tensor.matmul(ps2[:], wv, s2[kh][:],
                             start=(kh == 0), stop=(kh == 1))
        t = work.tile([128, BL], F32, tag=f"y{mh}")
        nc.scalar.activation(t[:], ps2[:], AF.Identity,
                             bias=btiles["bo2"][:, mh:mh + 1])
        y_parts.append(t)

    out_view = out_d.rearrange("b (mh o) -> mh o b", mh=2)
    for mh in range(2):
        nc.sync.dma_start(out_view[mh], y_parts[mh][:])

    ctx.close()


# ------------------------------------------------- wait-split fixup
# walrus in this toolchain rejects instructions with >2 sync waits; hoist
# excess waits onto no-ops inserted just before the offending instruction.


def _split_excess_waits(nc, cap=2):
    counter = [0]
    for fn in nc.m.functions:
        for blk in fn.blocks:
            insts = blk.instructions
            out = []
            changed = False
            for inst in insts:
                cap = 1
                si = inst.sync_info
                waits = list(si.on_wait) if si is not None else []
                if len(waits) > cap:
                    changed = True
                    extra, keep = waits[:-cap], waits[-cap:]
                    for j in range(0, len(extra), cap):
                        grp = extra[j:j + cap]
                        nop = mybir.InstNoOp(
                            name=f"wait-split-{counter[0]}", ins=[], outs=[])
                        counter[0] += 1
                        nop.engine = inst.engine
                        nop.sync_info = mybir.SyncInfo(on_wait=grp,
                                                       on_update=[])
                        out.append(nop)
                    inst.sync_info = mybir.SyncInfo(
                        on_wait=keep, on_update=list(si.on_update))
                out.append(inst)
            if changed:
                blk.instructions = out


# ------------------------------------------------- host-side packing


def _pack_params(p):
    """Build the device-layout parameter arrays (replicated per core)."""
    f32 = np.float32
    w_in = np.asarray(p["w_in"], f32)       # (8, 1)
    b_in = np.asarray(p["b_in"], f32)       # (8,)
    w_dil = np.asarray(p["w_dil"], f32)     # (NB, 8, 8, 2)
    b_dil = np.asarray(p["b_dil"], f32)     # (NB, 8)
    w_skip = np.asarray(p["w_skip"], f32)   # (NB, SK, 8)
    b_skip = np.asarray(p["b_skip"], f32)   # (NB, SK)
    w_f = np.asarray(p["w_f"], f32)
    b_f = np.asarray(p["b_f"], f32)
    w_g = np.asarray(p["w_g"], f32)
    b_g = np.asarray(p["b_g"], f32)
    w_out = np.asarray(p["w_out"], f32)
    b_out = np.asarray(p["b_out"], f32)
    w_o1 = np.asarray(p["w_o1"], f32)       # (SK, SK)
    b_o1 = np.asarray(p["b_o1"], f32)
    w_o2 = np.asarray(p["w_o2"], f32)
    b_o2 = np.asarray(p["b_o2"], f32)

    cvec = np.arange(128) % C               # channel per partition

    def strip_pack(mat_per_block, shifted=False):
        """mat_per_block: (NB, 8out, 8in) -> [128, NB*128] block-diag lhsT."""
        out = np.zeros((128, NB * 128), f32)
        for i in range(NB):
            for j in range(16):
                if shifted:
                    if j % 4 == 0:
                        continue
                    krow = 8 * (j - 1)
                else:
                    krow = 8 * j
                mcol = 128 * i + 8 * j
                # lhsT[k=in_c, m=out_o] = W[o, c]
                out[krow:krow + 8, mcol:mcol + 8] = mat_per_block[i].T
        return out

    def to_blocks(w):
        # [128, NB*128] -> [NB, 128, 128] contiguous per block
        return np.ascontiguousarray(
            w.reshape(128, NB, 128).transpose(1, 0, 2))

    wd1_h = to_blocks(strip_pack(w_dil[:, :, :, 1]))
    wd0_h = to_blocks(strip_pack(w_dil[:, :, :, 0]))
    wd0s_h = to_blocks(strip_pack(w_dil[:, :, :, 0], shifted=True))
    wf_h = to_blocks(strip_pack(w_f))
    wg_h = to_blocks(strip_pack(w_g))
    wo_h = to_blocks(strip_pack(w_out))

    win_h = np.zeros((BL * TCH, 128), f32)
    for j in range(BL * TCH):
        win_h[j, 8 * j:8 * j + 8] = w_in[:, 0]

    bin_h = b_in[cvec][:, None].astype(f32)
    bd_h = b_dil.T[cvec, :]                 # [128, NB]
    bf_h = b_f.T[cvec, :]
    bg_h = b_g.T[cvec, :]
    bo_h = b_out.T[cvec, :]

    # skip weights for transpose-contract: lhsT[c,mh][i, skm] = w_skip[i, 128mh+skm, c]
    wsk_h = np.zeros((NB, 16 * 128), f32)
    for c in range(C):
        for mh in range(2):
            wsk_h[:, (c * 2 + mh) * 128:(c * 2 + mh + 1) * 128] = \
                w_skip[:, 128 * mh:128 * (mh + 1), c]

    wo1_h4 = np.zeros((2, 2, 128, 128), f32)
    wo2_h4 = np.zeros((2, 2, 128, 128), f32)
    for kh in range(2):
        for mh in range(2):
            wo1_h4[kh, mh] = w_o1[128 * mh:128 * (mh + 1),
                                  128 * kh:128 * (kh + 1)].T
            wo2_h4[kh, mh] = w_o2[128 * mh:128 * (mh + 1),
                                  128 * kh:128 * (kh + 1)].T

    bsk_h = np.stack([b_skip.sum(0)[:128], b_skip.sum(0)[128:]], axis=1)
    bo1_h = np.stack([b_o1[:128], b_o1[128:]], axis=1)
    bo2_h = np.stack([b_o2[:128], b_o2[128:]], axis=1)

    # flatten [kh][mh] 128x128 chunks into [128, 4*128] (col-block order kh*2+mh)
    def flat4(w):
        return np.concatenate([w[kh, mh] for kh in range(2) for mh in range(2)],
                              axis=1)

    wo1_h = flat4(wo1_h4)
    wo2_h = flat4(wo2_h4)
    ident_h = np.eye(128, dtype=f32)

    return dict(win=win_h, wd1=wd1_h, wd0=wd0_h, wd0s=wd0s_h, wf=wf_h,
                wg=wg_h, wo=wo_h, bin=bin_h, bd=bd_h, bf=bf_h, bg=bg_h,
                bo=bo_h, wsk=wsk_h, ident=ident_h, bsk=bsk_h, wo1=wo1_h,
                bo1=bo1_h, wo2=wo2_h, bo2=bo2_h)


_NC_CACHE = {}


def get_nc(fixup=True):
    key = "nc" if fixup else "nc_nofix"
    if key not in _NC_CACHE:
        _NC_CACHE[key] = _build_nc(fixup=fixup)
    return _NC_CACHE[key]


def make_in_maps(**inputs):
    x = np.asarray(inputs["x"], np.float32)
    params = _pack_params(inputs)
    in_maps = []
    for k in range(N_CORES):
        shard = x[BL * k:BL * (k + 1)]                       # (4, 2048)
        x_l = shard.reshape(BL, TCH, L).reshape(BL * TCH, L).copy()
        m = {"x": x_l}
        m.update(params)
        in_maps.append(m)
    return in_maps


def kernel(**inputs):
    nc = get_nc()
    in_maps = make_in_maps(**inputs)
    res = run_bass_kernel_spmd(nc, in_maps, list(range(N_CORES)))
    outs = [res.results[k]["out"] for k in range(N_CORES)]
    return np.concatenate(outs, axis=0).astype(np.float32)


# revision 40
# speedup vs baseline: 1.7748x; 1.1107x over previous
"""WaveNet-like dense CNN on 8 TRN2 NeuronCores — batch data parallel.

Self-contained: hardcodes shapes from the problem spec.
  x: (32, 2048) f32 -> out: (32, 256) f32
Layout per core (4 batch samples): partitions p = b*32 + tc*8 + c
(b in 0..3 sample, tc in 0..3 time-chunk of 512, c in 0..7 channel),
free axis = t_low in 0..511.  All 1x1 convs are block-diagonal 64x64
strip matmuls (2 concurrent tile_position strips).  The dilated causal
conv is 3 matmuls (full tap1, shifted-view tap0, cross-chunk boundary
tap0 with a partition-shifting weight) — no data movement for shifts.
Only skip[:, :, -1] is ever used, so skip convs run on 1 column/block.
"""

import sys

sys.path.insert(0, "/opt/trn_rl_repo")

import numpy as np
import ml_dtypes

import concourse.bass as bass
import concourse.tile as tile
from concourse import mybir
from concourse.bass_utils import run_bass_kernel_spmd

F32 = mybir.dt.float32
F32R = mybir.dt.float32r
BF16 = mybir.dt.bfloat16
AF = mybir.ActivationFunctionType
ALU = mybir.AluOpType

SEQ_LEN = 2048
C = 8
SK = 256
NB = 32
B = 32
N_CORES = 8
BL = B // N_CORES      # 4 samples per core
TCH = 4                # time chunks per sample
L = SEQ_LEN // TCH     # 512 free elems
NSTRIP = 2             # two 64x64 tile_position strips
SW = 64                # strip width (partitions/outputs per strip)
GPS = 8                # groups per strip (8 groups of 8 channels)


def _dil(i):
    return 2 ** (i % 8)


# ---------------------------------------------------------------- build


def _build_nc(fixup=True):
    nc = bass.Bass("TRN2", target_bir_lowering=False, debug=False,
                   num_devices=N_CORES)

    def din(name, shape, dt=F32R):
        return nc.dram_tensor(name, shape, dt, kind="ExternalInput").ap()

    x_d = din("x", [BL * TCH, L])
    win_d = din("win", [BL * TCH, 128])
    # [NB, 128, 128]: contiguous per block so the per-block DMA reads
    # DRAM sequentially (full burst bandwidth)
    wd1_d = din("wd1", [NB, 128, 128])
    wd0_d = din("wd0", [NB, 128, 128])
    wd0s_d = din("wd0s", [NB, 128, 128])
    wf_d = din("wf", [NB, 128, 128], BF16)
    wg_d = din("wg", [NB, 128, 128], BF16)
    wo_d = din("wo", [NB, 128, 128], BF16)
    bin_d = din("bin", [128, 1], F32)
    bd_d = din("bd", [128, NB], F32)
    bf_d = din("bf", [128, NB], F32)
    bg_d = din("bg", [128, NB], F32)
    bo_d = din("bo", [128, NB], F32)
    wsk_d = din("wsk", [32, 16 * 128])
    ident_d = din("ident", [128, 128])
    bsk_d = din("bsk", [128, 2], F32)
    wo1_d = din("wo1", [128, 4 * 128])
    bo1_d = din("bo1", [128, 2], F32)
    wo2_d = din("wo2", [128, 4 * 128])
    bo2_d = din("bo2", [128, 2], F32)

    out_d = nc.dram_tensor("out", [BL, SK], F32, kind="ExternalOutput").ap()

    with tile.TileContext(nc) as tc:
        _emit(nc, tc, x_d, win_d, wd1_d, wd0_d, wd0s_d, wf_d, wg_d, wo_d,
              bin_d, bd_d, bf_d, bg_d, bo_d, wsk_d, ident_d, bsk_d,
              wo1_d, bo1_d, wo2_d, bo2_d, out_d)

    if fixup:
        _split_excess_waits(nc)
    return nc


def _emit(nc, tc, x_d, win_d, wd1_d, wd0_d, wd0s_d, wf_d, wg_d, wo_d,
          bin_d, bd_d, bf_d, bg_d, bo_d, wsk_d, ident_d, bsk_d,
          wo1_d, bo1_d, wo2_d, bo2_d, out_d):
    from contextlib import ExitStack
    ctx = ExitStack()
    const = ctx.enter_context(tc.tile_pool(name="const", bufs=1))
    work = ctx.enter_context(tc.tile_pool(name="work", bufs=2))
    hpool = ctx.enter_context(tc.tile_pool(name="h", bufs=3))
    pspool = ctx.enter_context(tc.tile_pool(name="ps", bufs=3, space="PSUM"))
    pspool_f = ctx.enter_context(tc.tile_pool(name="psf", bufs=1, space="PSUM"))
    pspool_g = ctx.enter_context(tc.tile_pool(name="psg", bufs=1, space="PSUM"))
    pspool_o = ctx.enter_context(tc.tile_pool(name="pso", bufs=2, space="PSUM"))
    pspool1 = ctx.enter_context(tc.tile_pool(name="ps1", bufs=1, space="PSUM"))

    # ---- persistent loads (small params first so compute starts early;
    # bulk per-block weights stream in behind and hide under compute)
    x_sb = const.tile([BL * TCH, L], F32R, tag="x")
    nc.sync.dma_start(x_sb[:], x_d[:])
    win_sb = const.tile([BL * TCH, 128], F32R, tag="win")
    nc.sync.dma_start(win_sb[:], win_d[:])

    btiles = {}
    for nm, d, w in (("bin", bin_d, 1), ("bd", bd_d, NB), ("bf", bf_d, NB),
                     ("bg", bg_d, NB), ("bo", bo_d, NB), ("bsk", bsk_d, 2),
                     ("bo1", bo1_d, 2), ("bo2", bo2_d, 2)):
        t = const.tile([128, w], F32, tag=nm)
        nc.sync.dma_start(t[:], d[:])
        btiles[nm] = t

    wsk_sb = const.tile([32, 16 * 128], F32R, tag="wsk")
    nc.sync.dma_start(wsk_sb[:], wsk_d[:])
    ident_sb = const.tile([128, 128], F32R, tag="ident")
    nc.sync.dma_start(ident_sb[:], ident_d[:])
    wo1_sb = const.tile([128, 4 * 128], F32R, tag="wo1")
    nc.sync.dma_start(wo1_sb[:], wo1_d[:])
    wo2_sb = const.tile([128, 4 * 128], F32R, tag="wo2")
    nc.sync.dma_start(wo2_sb[:], wo2_d[:])

    wtiles = {}
    wnames = (("wd1", wd1_d), ("wd0", wd0_d), ("wd0s", wd0s_d),
              ("wf", wf_d), ("wg", wg_d), ("wo", wo_d))
    for nm, _ in wnames:
        t = const.tile([128, NB * 128],
                       BF16 if nm in ("wf", "wg", "wo") else F32R, tag=nm)
        wtiles[nm] = t
    # per-block DMA order: all six weights of block 0 first, then block 1...
    for i in range(NB):
        for nm, d in wnames:
            nc.sync.dma_start(wtiles[nm][:, 128 * i:128 * (i + 1)], d[i])

    s_sb = const.tile([128, NB], F32R, tag="scap")

    # ---- input 1x1 conv: h0 = w_in * x + b_in  (K=16 matmul broadcast)
    ps_h = pspool.tile([128, L], F32, tag="ps")
    nc.tensor.matmul(ps_h[:], win_sb[:], x_sb[:], start=True, stop=True)
    h_cur = hpool.tile([128, L], F32R, tag="h")
    nc.scalar.activation(h_cur[:], ps_h[:], AF.Identity,
                         bias=btiles["bin"][:, 0:1])

    # Receptive-field window: block i only needs output cols [c0[i], 512).
    # Only skip[:, :, -1] is used, so work shrinks down the stack.
    c0 = [0] * NB
    need = 511
    for i in reversed(range(NB)):
        c0[i] = max(0, need & ~1)
        need = max(0, c0[i] - _dil(i))

    # ---- 32 residual blocks, each split into 2 column chunks that
    # software-pipeline across engines (chunk B of block i overlaps
    # chunk A of block i, and blocks overlap each other).
    for i in range(NB):
        d = _dil(i)
        wc = slice(128 * i, 128 * (i + 1))
        o0 = c0[i]
        mid = ((o0 + L) // 2) & ~1
        chunks = [(o0, mid), (mid, L)] if mid - o0 >= 2 else [(o0, L)]
        last = i == NB - 1   # h_32 is never used: only the capture col

        ps_a = pspool.tile([128, L], F32, tag="ps")
        for (a0, a1) in chunks:
            # tap1 over [a0, a1)
            nc.tensor.matmul(ps_a[:, a0:a1], wtiles["wd1"][:, wc],
                             h_cur[:, a0:a1], start=True, stop=False,
                             skip_group_check=True)
            i0 = max(a0, 2 if d == 1 else d)
            bnd = a0 < d and d > 1
            # tap0 intra: out [i0, a1) <- h [i0-d, a1-d)
            nc.tensor.matmul(ps_a[:, i0:a1], wtiles["wd0"][:, wc],
                             h_cur[:, i0 - d:a1 - d], start=False,
                             stop=not bnd, skip_group_check=True)
            if bnd:
                # cross-chunk boundary: out [a0, d) <- prev tc-chunk tail
                nc.tensor.matmul(ps_a[:, a0:d], wtiles["wd0s"][:, wc],
                                 h_cur[:, L - d + a0:L], start=False,
                                 stop=True, skip_group_check=True)
        if d == 1 and o0 < 2:
            # fp32r MMs need even/aligned column ranges; patch cols 0,1.
            ps_bc = pspool.tile([128, 4], F32, tag="ps")
            nc.tensor.matmul(ps_bc[:, 0:2], wtiles["wd0s"][:, wc],
                             h_cur[:, L - 2:L], start=True, stop=True,
                             skip_group_check=True)
            nc.tensor.matmul(ps_bc[:, 2:4], wtiles["wd0"][:, wc],
                             h_cur[:, 0:2], start=True, stop=True,
                             skip_group_check=True)
            fix_sb = work.tile([128, 2], F32, tag="fix")
            nc.scalar.copy(fix_sb[:, 0:1], ps_bc[:, 1:2])
            nc.scalar.copy(fix_sb[:, 1:2], ps_bc[:, 2:3])
            nc.vector.tensor_tensor(ps_a[:, 0:2], ps_a[:, 0:2],
                                    fix_sb[:], op=ALU.add)

        # capture relu(a)[:, -1] for the skip head
        nc.vector.tensor_scalar(s_sb[:, i:i + 1], ps_a[:, L - 1:L],
                                btiles["bd"][:, i:i + 1], 0.0,
                                op0=ALU.add, op1=ALU.max)
        if last:
            break

        a_sb = work.tile([128, L], BF16, tag="a")
        ps_f = pspool_f.tile([128, L], F32, tag="psf")
        ps_g = pspool_g.tile([128, L], F32, tag="psg")
        ps_o = pspool_o.tile([128, L], F32, tag="pso")
        f_sb = work.tile([128, L], BF16, tag="f")
        g_sb = work.tile([128, L], BF16, tag="g")
        fg_sb = work.tile([128, L], BF16, tag="fg")
        h_new = hpool.tile([128, L], F32R, tag="h")
        for k, (a0, a1) in enumerate(chunks):
            cs = slice(a0, a1)
            # a = relu(psum + b_dil); alternate engines per chunk
            if k == 0:
                nc.scalar.activation(a_sb[:, cs], ps_a[:, cs], AF.Relu,
                                     bias=btiles["bd"][:, i:i + 1])
            else:
                nc.vector.tensor_scalar(a_sb[:, cs], ps_a[:, cs],
                                        btiles["bd"][:, i:i + 1], 0.0,
                                        op0=ALU.add, op1=ALU.max)
            nc.tensor.matmul(ps_f[:, cs], wtiles["wf"][:, wc], a_sb[:, cs],
                             start=True, stop=True, skip_group_check=True)
            nc.tensor.matmul(ps_g[:, cs], wtiles["wg"][:, wc], a_sb[:, cs],
                             start=True, stop=True, skip_group_check=True)
            nc.scalar.activation(f_sb[:, cs], ps_f[:, cs], AF.Tanh,
                                 bias=btiles["bf"][:, i:i + 1])
            nc.scalar.activation(g_sb[:, cs], ps_g[:, cs], AF.Sigmoid,
                                 bias=btiles["bg"][:, i:i + 1])
            nc.vector.tensor_tensor(fg_sb[:, cs], f_sb[:, cs], g_sb[:, cs],
                                    op=ALU.mult)
            nc.tensor.matmul(ps_o[:, cs], wtiles["wo"][:, wc], fg_sb[:, cs],
                             start=True, stop=True, skip_group_check=True)
            # h_new = (ps_o + b_out) + h_cur
            if k == 0:
                nc.scalar.activation(h_new[:, cs], ps_o[:, cs], AF.Identity,
                                     bias=btiles["bo"][:, i:i + 1])
                nc.vector.tensor_tensor(h_new[:, cs], h_new[:, cs],
                                        h_cur[:, cs], op=ALU.add)
            else:
                nc.vector.tensor_scalar_add(h_new[:, cs], ps_o[:, cs],
                                            btiles["bo"][:, i:i + 1])
                nc.vector.tensor_tensor(h_new[:, cs], h_new[:, cs],
                                        h_cur[:, cs], op=ALU.add)
        h_cur = h_new

    # ---- skip head: PE-transpose S [128, NB] -> [NB, 128], then contract
    # over blocks i with per-channel strided-free matmuls.
    ps_t = pspool1.tile([NB, 128], F32R, tag="pst")
    nc.tensor.transpose(ps_t[:], s_sb[:], ident_sb[:])
    st_sb = work.tile([NB, 128], F32R, tag="st")
    nc.scalar.copy(st_sb[:], ps_t[:])

    y_parts = []
    s1 = []
    ps_sk = [pspool.tile([128, BL], F32, tag="ps", name=f"ps_sk{_m}")
             for _m in range(2)]
    for mh in range(2):
        for c in range(C):
            wv = wsk_sb[:, (c * 2 + mh) * 128:(c * 2 + mh + 1) * 128]
            # rhs: st_sb[i, 24 + c + 32*b] for b in 0..3 (free stride 32)
            rv = st_sb[:, 24 + c::32]
            nc.tensor.matmul(ps_sk[mh][:], wv, rv,
                             start=(c == 0), stop=(c == C - 1))
    for mh in range(2):
        t = work.tile([128, BL], F32R, tag=f"s1_{mh}")
        nc.scalar.activation(t[:], ps_sk[mh][:], AF.Relu,
                             bias=btiles["bsk"][:, mh:mh + 1])
        s1.append(t)
    s2 = []
    for mh in range(2):
        ps1 = pspool.tile([128, BL], F32, tag="ps")
        for kh in range(2):
            wv = wo1_sb[:, (kh * 2 + mh) * 128:(kh * 2 + mh + 1) * 128]
            nc.tensor.matmul(ps1[:], wv, s1[kh][:],
                             start=(kh == 0), stop=(kh == 1))
        t = work.tile([128, BL], F32R, tag=f"s2_{mh}")
        nc.scalar.activation(t[:], ps1[:], AF.Relu,
                             bias=btiles["bo1"][:, mh:mh + 1])
        s2.append(t)
    for mh in range(2):
        ps2 = pspool.tile([128, BL], F32, tag="ps")
        for kh in range(2):
            wv = wo2_sb[:, (kh * 2 + mh) * 128:(kh * 2 + mh + 1) * 128]
            nc.tensor.matmul(ps2[:], wv, s2[kh][:],
                             start=(kh == 0), stop=(kh == 1))
        t = work.tile([128, BL], F32, tag=f"y{mh}")
        nc.scalar.activation(t[:], ps2[:], AF.Identity,
                             bias=btiles["bo2"][:, mh:mh + 1])
        y_parts.append(t)

    out_view = out_d.rearrange("b (mh o) -> mh o b", mh=2)
    for mh in range(2):
        nc.sync.dma_start(out_view[mh], y_parts[mh][:])

    ctx.close()


# ------------------------------------------------- wait-split fixup
# walrus in this toolchain rejects instructions with >2 sync waits; hoist
# excess waits onto no-ops inserted just before the offending instruction.


def _split_excess_waits(nc, cap=2):
    counter = [0]
    for fn in nc.m.functions:
        for blk in fn.blocks:
            insts = blk.instructions
            out = []
            changed = False
            for inst in insts:
                cap = 1
                si = inst.sync_info
                waits = list(si.on_wait) if si is not None else []
                if len(waits) > cap:
                    changed = True
                    extra, keep = waits[:-cap], waits[-cap:]
                    for j in range(0, len(extra), cap):
                        grp = extra[j:j + cap]
                        nop = mybir.InstNoOp(
                            name=f"wait-split-{counter[0]}", ins=[], outs=[])
                        counter[0] += 1
                        nop.engine = inst.engine
                        nop.sync_info = mybir.SyncInfo(on_wait=grp,
                                                       on_update=[])
                        out.append(nop)
                    inst.sync_info = mybir.SyncInfo(
                        on_wait=keep, on_update=list(si.on_update))
                out.append(inst)
            if changed:
                blk.instructions = out


# ------------------------------------------------- host-side packing


def _pack_params(p):
    """Build the device-layout parameter arrays (replicated per core)."""
    f32 = np.float32
    w_in = np.asarray(p["w_in"], f32)       # (8, 1)
    b_in = np.asarray(p["b_in"], f32)       # (8,)
    w_dil = np.asarray(p["w_dil"], f32)     # (NB, 8, 8, 2)
    b_dil = np.asarray(p["b_dil"], f32)     # (NB, 8)
    w_skip = np.asarray(p["w_skip"], f32)   # (NB, SK, 8)
    b_skip = np.asarray(p["b_skip"], f32)   # (NB, SK)
    w_f = np.asarray(p["w_f"], f32)
    b_f = np.asarray(p["b_f"], f32)
    w_g = np.asarray(p["w_g"], f32)
    b_g = np.asarray(p["b_g"], f32)
    w_out = np.asarray(p["w_out"], f32)
    b_out = np.asarray(p["b_out"], f32)
    w_o1 = np.asarray(p["w_o1"], f32)       # (SK, SK)
    b_o1 = np.asarray(p["b_o1"], f32)
    w_o2 = np.asarray(p["w_o2"], f32)
    b_o2 = np.asarray(p["b_o2"], f32)

    cvec = np.arange(128) % C               # channel per partition

    def strip_pack(mat_per_block, shifted=False):
        """mat_per_block: (NB, 8out, 8in) -> [128, NB*128] block-diag lhsT."""
        out = np.zeros((128, NB * 128), f32)
        for i in range(NB):
            for j in range(16):
                if shifted:
                    if j % 4 == 0:
                        continue
                    krow = 8 * (j - 1)
                else:
                    krow = 8 * j
                mcol = 128 * i + 8 * j
                # lhsT[k=in_c, m=out_o] = W[o, c]
                out[krow:krow + 8, mcol:mcol + 8] = mat_per_block[i].T
        return out

    def to_blocks(w):
        # [128, NB*128] -> [NB, 128, 128] contiguous per block
        return np.ascontiguousarray(
            w.reshape(128, NB, 128).transpose(1, 0, 2))

    wd1_h = to_blocks(strip_pack(w_dil[:, :, :, 1]))
    wd0_h = to_blocks(strip_pack(w_dil[:, :, :, 0]))
    wd0s_h = to_blocks(strip_pack(w_dil[:, :, :, 0], shifted=True))
    wf_h = to_blocks(strip_pack(w_f)).astype(ml_dtypes.bfloat16)
    wg_h = to_blocks(strip_pack(w_g)).astype(ml_dtypes.bfloat16)
    wo_h = to_blocks(strip_pack(w_out)).astype(ml_dtypes.bfloat16)

    win_h = np.zeros((BL * TCH, 128), f32)
    for j in range(BL * TCH):
        win_h[j, 8 * j:8 * j + 8] = w_in[:, 0]

    bin_h = b_in[cvec][:, None].astype(f32)
    bd_h = b_dil.T[cvec, :]                 # [128, NB]
    bf_h = b_f.T[cvec, :]
    bg_h = b_g.T[cvec, :]
    bo_h = b_out.T[cvec, :]

    # skip weights for transpose-contract: lhsT[c,mh][i, skm] = w_skip[i, 128mh+skm, c]
    wsk_h = np.zeros((NB, 16 * 128), f32)
    for c in range(C):
        for mh in range(2):
            wsk_h[:, (c * 2 + mh) * 128:(c * 2 + mh + 1) * 128] = \
                w_skip[:, 128 * mh:128 * (mh + 1), c]

    wo1_h4 = np.zeros((2, 2, 128, 128), f32)
    wo2_h4 = np.zeros((2, 2, 128, 128), f32)
    for kh in range(2):
        for mh in range(2):
            wo1_h4[kh, mh] = w_o1[128 * mh:128 * (mh + 1),
                                  128 * kh:128 * (kh + 1)].T
            wo2_h4[kh, mh] = w_o2[128 * mh:128 * (mh + 1),
                                  128 * kh:128 * (kh + 1)].T

    bsk_h = np.stack([b_skip.sum(0)[:128], b_skip.sum(0)[128:]], axis=1)
    bo1_h = np.stack([b_o1[:128], b_o1[128:]], axis=1)
    bo2_h = np.stack([b_o2[:128], b_o2[128:]], axis=1)

    # flatten [kh][mh] 128x128 chunks into [128, 4*128] (col-block order kh*2+mh)
    def flat4(w):
        return np.concatenate([w[kh, mh] for kh in range(2) for mh in range(2)],
                              axis=1)

    wo1_h = flat4(wo1_h4)
    wo2_h = flat4(wo2_h4)
    ident_h = np.eye(128, dtype=f32)

    return dict(win=win_h, wd1=wd1_h, wd0=wd0_h, wd0s=wd0s_h, wf=wf_h,
                wg=wg_h, wo=wo_h, bin=bin_h, bd=bd_h, bf=bf_h, bg=bg_h,
                bo=bo_h, wsk=wsk_h, ident=ident_h, bsk=bsk_h, wo1=wo1_h,
                bo1=bo1_h, wo2=wo2_h, bo2=bo2_h)


_NC_CACHE = {}


def get_nc(fixup=True):
    key = "nc" if fixup else "nc_nofix"
    if key not in _NC_CACHE:
        _NC_CACHE[key] = _build_nc(fixup=fixup)
    return _NC_CACHE[key]


def make_in_maps(**inputs):
    x = np.asarray(inputs["x"], np.float32)
    params = _pack_params(inputs)
    in_maps = []
    for k in range(N_CORES):
        shard = x[BL * k:BL * (k + 1)]                       # (4, 2048)
        x_l = shard.reshape(BL, TCH, L).reshape(BL * TCH, L).copy()
        m = {"x": x_l}
        m.update(params)
        in_maps.append(m)
    return in_maps


def kernel(**inputs):
    nc = get_nc()
    in_maps = make_in_maps(**inputs)
    res = run_bass_kernel_spmd(nc, in_maps, list(range(N_CORES)))
    outs = [res.results[k]["out"] for k in range(N_CORES)]
    return np.concatenate(outs, axis=0).astype(np.float32)
